# revision 42
# baseline (speedup 1.0000x reference)
"""Trainium2 Bass kernel for nn_Attn_47768626266275.

Computation (reference):
    energy[b,s,:] = W @ enc[b,s,:] + bias          # nn.Linear
    scores[b,s]   = hidden[b,:] . energy[b,s,:]
    out           = softmax(scores, axis=-1)[:, None, :]

Algebraic rewrite used here:
    scores[b,s] = enc[b,s,:] . v[b,:] + c[b],  v = hidden @ W,  c = hidden . bias
    softmax is shift-invariant along s, so c[b] drops out entirely.

This turns the [B*S,H]x[H,H] projection (137 GFLOP) into a [B,H]x[H,H] matmul
plus a streamed per-row dot product -> the kernel is HBM-bound on reading
encoder_outputs exactly once (33.5 MB/core across 8 cores).

Precision: the dot-product stream runs in fp16 (measured end-to-end max rel
err 2.6e-3 vs the fp32 reference, against a 2e-2 gate). enc is cast
f32->fp16 inline by the SWDGE DMA engines during the HBM->SBUF load, which
halves DVE time (tensor_tensor-class ops get the 2x_1p packed mode only for
16-bit dtypes) and halves SBUF footprint; HBM read traffic is unchanged.
The accumulator (scores) stays fp32.

Sharding: data-parallel over batch. Core i handles batches [4i, 4i+4).
No collectives (a W-shard + ReduceScatter of partial v was evaluated and
rejected: the collective's latency sits on the vb critical path that gates
all compute, costing more than the 1.75 MB of W traffic it saves).

Per-core pipeline:
  - SWDGE (gpsimd ring): W fp16 first (same-queue FIFO lands its 2 MB
    ahead of the enc bytes; on the HWDGE ring W gets starved to ~120-190
    GB/s by SWDGE packet competition and vb, which gates ALL compute,
    arrives ~25 us late), then the enc stream in [128, 4, 1024]
    supertiles with inline f32->fp16 cast. Nothing with a cross-engine
    wait may sit on the Pool queue while the stream is live - it stalls
    descriptor generation and with it the whole HBM stream.
  - HWDGE (sync ring): hiddenT, the first supertile as f32 (an all-STT
    f32 path computes it while the Pool queue spools up); output stores.
  - PE: v = hiddenT.T @ W (fp16 in, f32 PSUM); per-batch broadcast of
    v[b] to all 128 partitions via a one-hot stationary matrix.
  - Dot products, hybrid across engines (measured per-[128,1024]-tile:
    STT w/ fused accum = ~1220 ns DVE regardless of dtype - every
    accum-bearing DVE op is capped at 1x mode; plain TT fp16 hits 2x =
    ~690 ns; ACT Copy-with-accum = ~1150+280 ns). Neither engine alone
    keeps up with the ~94 us stream, so ~9/16 columns per batch run
    TT(DVE) + Copy-accum(ACT) and ~7/16 run STT(DVE), putting both
    engines at ~75 us. tensor_tensor_reduce would fuse at 2x on paper
    but wedges the core on this HW (custom-DVE op, no 2x uop either).
  - softmax over the [128, 16] f32 score tile per batch: while the
    stream is live, partition reductions go through PE transpose/
    ones-matmul round-trips (keeps the Pool queue clean); the final
    batch - whose softmax is the exposed serial tail - uses single
    gpsimd partition_all_reduce ops instead, roughly halving the chain.
    The last supertile's DMA is split in half so its tiles land early.

Single-run HW times vary up to ~25% with neighbor traffic on the shared
device; judge changes by the min over >=3 runs (best seen: ~106.7 us,
from a 131 us starting point; floor is ~100: 35.6 MB at ~375 GB/s + ~6.5
us fixed engine-init preamble + ~4 us tail).
"""

import numpy as np

import concourse.bass as bass
import concourse.bacc as bacc
import concourse.tile as tile
from concourse import mybir
from concourse.masks import make_identity

B = 32          # full batch
S = 2048        # sequence
H = 1024        # hidden
NCORES = 8
BPC = B // NCORES   # batches per core = 4
NU = 4          # supertiles per batch (1 MB fp16 each)
NT = 4          # 128-row subtiles per supertile
NC_P = 128      # partitions
KCH = H // NC_P  # 8 contraction chunks for the v matmul

F32 = mybir.dt.float32
F16 = mybir.dt.float16

_CACHED = {}


def _build_bass():
    from contextlib import ExitStack

    nc = bacc.Bacc()

    enc_h = nc.declare_dram_parameter("enc", [BPC, S, H], F32, isOutput=False)
    # hTp[p, k*BPC + b] = hidden[b, k*128 + p] — one contiguous row per
    # partition so the DMA is 128 fat descriptors instead of 1024 tiny ones
    hT_h = nc.declare_dram_parameter("hTp", [NC_P, KCH * BPC], F16, isOutput=False)
    w_h = nc.declare_dram_parameter("W", [H, H], F16, isOutput=False)
    out_h = nc.declare_dram_parameter("out", [BPC, S], F32, isOutput=True)

    with tile.TileContext(nc) as tc, ExitStack() as ctx:
        _emit(ctx, tc, enc_h, hT_h, w_h, out_h)
    return nc


def _emit(ctx, tc, enc_h, hT_h, w_h, out_h):
    nc = tc.nc

    singles = ctx.enter_context(tc.tile_pool(name="singles", bufs=1))
    wchunks = ctx.enter_context(tc.tile_pool(name="wchunks", bufs=8))
    encp = ctx.enter_context(tc.tile_pool(name="encp", bufs=8))
    scratchp = ctx.enter_context(tc.tile_pool(name="scratchp", bufs=3))
    prodp = ctx.enter_context(tc.tile_pool(name="prodp", bufs=5))
    scoresp = ctx.enter_context(tc.tile_pool(name="scoresp", bufs=3))
    smallp = ctx.enter_context(tc.tile_pool(name="smallp", bufs=4))
    pmm = ctx.enter_context(tc.tile_pool(name="pmm", bufs=2, space="PSUM"))
    psmall = ctx.enter_context(tc.tile_pool(name="psmall", bufs=1, space="PSUM"))

    # ---- prefetch the first two enc supertiles ---------------------------
    # issued before any other Pool-queue work (constants below run on
    # gpsimd) so SWDGE descriptor generation — and with it the HBM
    # stream — starts at t~0 instead of ~8 us in
    enc_ap = enc_h[:].rearrange("b (u t p) h -> b u p t h", u=NU, t=NT, p=NC_P)
    # W rides the SWDGE/Pool queue AHEAD of the enc stream. W gates vb which
    # gates all compute, and on the HWDGE ring it gets starved down to
    # ~120-190 GB/s by SWDGE packet competition (compute then starts ~30 us
    # in). Same-queue FIFO guarantees W's 2 MB lands before any enc byte.
    # These are plain fp16 loads (no cast) with no cross-engine waits, so
    # they cannot stall descriptor generation.
    w_ap = w_h[:].rearrange("(k p) h -> k p h", p=NC_P)
    w_sbs = []
    for k in range(KCH):
        w_sb = wchunks.tile([NC_P, H], F16, tag="w", name="w_sb")
        nc.gpsimd.dma_start(out=w_sb, in_=w_ap[k])
        w_sbs.append(w_sb)
    prefetched = []
    for u in range(1, 3):
        e_sb = encp.tile([NC_P, NT, H], F16, tag="enc", name="e_sb")
        nc.gpsimd.dma_start(out=e_sb, in_=enc_ap[0, u])
        prefetched.append(e_sb)

    # ---- constants -------------------------------------------------------
    ident = singles.tile([NC_P, NC_P], F32, tag="ident")
    make_identity(nc, ident)
    ones_col = singles.tile([1, NC_P], F32, tag="ones_col")   # lhsT for bcast
    nc.vector.memset(ones_col, 1.0)
    ones_sum = singles.tile([NC_P, 1], F32, tag="ones_sum")   # rhs for P-sum
    nc.vector.memset(ones_sum, 1.0)
    # junk tile for PE warmup — DVE memset, ready ~4 us before make_identity
    # (which sits behind the Q7 preamble + prefetch descriptor generation)
    warm_in = singles.tile([NC_P, NC_P], F32, tag="warm_in")
    nc.vector.memset(warm_in, 0.0)
    # sel[:, b, :] is a [BPC, 128] stationary matrix whose row b is all-ones:
    # matmul(lhsT=sel[:,b,:], rhs=v_sb) broadcasts v[b,:] to all partitions.
    sel = singles.tile([BPC, BPC, NC_P], F16, tag="sel")
    nc.gpsimd.memset(sel, 0.0)
    nc.gpsimd.affine_select(
        out=sel,
        in_=sel,
        compare_op=mybir.AluOpType.not_equal,
        fill=1.0,
        base=0,
        # expr = p - b  -> fill 1.0 where p == b
        pattern=[[-1, BPC], [0, NC_P]],
        channel_multiplier=1,
    )

    # ---- PE warmup: ~3.5 us of junk matmuls so the HAM clock-gate opens
    # (1.2 -> 2.4 GHz) before the v-chain matmuls arrive
    warm_ps = pmm.tile([NC_P, NC_P], F32, tag="mm", name="warm_ps")
    for _ in range(8):
        nc.tensor.matmul(warm_ps, lhsT=warm_in, rhs=warm_in, start=True, stop=True)

    # ---- load packed hiddenT (tiny, first on the sync ring) -------------
    hT_sb = singles.tile([NC_P, KCH, BPC], F16, tag="hT_sb")
    nc.sync.dma_start(
        out=hT_sb, in_=hT_h[:].rearrange("p (k b) -> p k b", b=BPC)
    )

    # ---- first supertile (b0,u0) rides the now-idle HWDGE ring as f32 ---
    # with W moved to the Pool queue, HWDGE has nothing but hT + output
    # stores; these 2 MB stream concurrently with W and come off the tail
    # of the SWDGE stream. The 4 tiles use an all-STT f32 path (f32 STT
    # costs the same ~1220 ns as fp16).
    e32_sb = singles.tile([NC_P, NT, H], F32, tag="e32_sb")
    nc.sync.dma_start(out=e32_sb, in_=enc_ap[0, 0])
    scratch32 = singles.tile([NC_P, H], F32, tag="scratch32")

    # ---- v = hiddenT.T @ W, chunk matmuls chase the W chunk arrivals ----
    v_ps = pmm.tile([BPC, H], F32, tag="mm")
    for k in range(KCH):
        for half in range(2):
            cols = slice(half * 512, (half + 1) * 512)
            nc.tensor.matmul(
                v_ps[:, cols],
                lhsT=hT_sb[:, k, :],
                rhs=w_sbs[k][:, cols],
                start=(k == 0),
                stop=(k == KCH - 1),
            )
    v_sb = singles.tile([BPC, H], F16, tag="v_sb")
    nc.scalar.copy(v_sb, v_ps)

    # ---- broadcast v[b] across all 128 partitions -----------------------
    vb_sb = []
    for b in range(BPC):
        vb_ps = pmm.tile([NC_P, H], F32, tag="mm")
        for half in range(2):
            cols = slice(half * 512, (half + 1) * 512)
            nc.tensor.matmul(
                vb_ps[:, cols],
                lhsT=sel[:, b, :],
                rhs=v_sb[:, cols],
                start=True,
                stop=True,
            )
        t = singles.tile([NC_P, H], F16, tag=f"vb{b}")
        nc.scalar.copy(t, vb_ps)
        vb_sb.append(t)
        if b == 0:
            vb0_32 = singles.tile([NC_P, H], F32, tag="vb0_32")
            nc.scalar.copy(vb0_32, vb_ps)

    # ---- main stream: scores + softmax ----------------------------------
    out_ap = out_h[:].rearrange("b (c p) -> b c p", p=NC_P)  # c = u*NT + t
    ncols = NU * NT
    # ACT's main output for the accumulate pass; the data is discarded
    # (only accum_out matters) so one buffer is enough — ACT is in-order
    sink = singles.tile([NC_P, H], F16, tag="sink")

    from concourse.tile import add_dep_helper

    def _pin(op, pin):
        # order a softmax DVE op after the given STT in the DVE stream so the
        # in-order DVE never idles on the op's cross-engine dependencies
        if pin is not None:
            add_dep_helper(op.ins, pin.ins, sync=False,
                           reason="defer softmax DVE op behind STT stream")

    from concourse import bass_isa

    # Two softmax implementations. The PE-based one is used while the enc
    # stream is live: its cross-engine round-trips hide under the stream and
    # it keeps the gpsimd/Pool queue free (any cross-engine wait there stalls
    # SWDGE descriptor generation and with it the whole HBM stream). The
    # gpsimd-based one collapses the partition reductions into single Q7 ops
    # and is used only for the final batch, after the last enc DMA has been
    # issued — it roughly halves the serial tail chain.

    def pe_softmax_stage1(st, pin=None):
        # row-max over the 16 score columns, transpose to one partition
        st["rmax"] = smallp.tile([NC_P, 1], F32, tag="rmax", name="rmax")
        _pin(
            nc.vector.tensor_reduce(
                out=st["rmax"], in_=st["scores"], axis=mybir.AxisListType.X,
                op=mybir.AluOpType.max,
            ),
            pin,
        )
        rmaxT_ps = psmall.tile([1, NC_P], F32, tag="ps_a", name="rmaxT_ps")
        nc.tensor.transpose(rmaxT_ps, st["rmax"], ident)
        st["rmaxT"] = smallp.tile([1, NC_P], F32, tag="rmaxT", name="rmaxT")
        nc.scalar.copy(st["rmaxT"], rmaxT_ps)

    def pe_softmax_stage2(st, pin=None):
        # global max -> -max on all partitions -> exp with accumulate -> sum
        gmax = smallp.tile([1, 1], F32, tag="gmax", name="gmax")
        _pin(
            nc.vector.tensor_reduce(
                out=gmax, in_=st["rmaxT"], axis=mybir.AxisListType.X,
                op=mybir.AluOpType.max,
            ),
            pin,
        )
        gmax_ps = psmall.tile([NC_P, 1], F32, tag="ps_b", name="gmax_ps")
        nc.tensor.matmul(gmax_ps, lhsT=ones_col, rhs=gmax, start=True, stop=True)
        negmax = smallp.tile([NC_P, 1], F32, tag="negmax", name="negmax")
        nc.scalar.mul(negmax, gmax_ps, -1.0)
        st["probs"] = scoresp.tile([NC_P, ncols], F32, tag="probs", name="probs")
        ssum = smallp.tile([NC_P, 1], F32, tag="ssum", name="ssum")
        nc.scalar.activation(
            out=st["probs"], in_=st["scores"],
            func=mybir.ActivationFunctionType.Exp,
            bias=negmax, scale=1.0, accum_out=ssum,
        )
        tot_ps = psmall.tile([1, 1], F32, tag="ps_c", name="tot_ps")
        nc.tensor.matmul(tot_ps, lhsT=ssum, rhs=ones_sum, start=True, stop=True)
        st["tot_ps"] = tot_ps

    def pe_softmax_stage3(st, pin=None):
        # 1/sum, broadcast, transposed normalize, store
        rinv = smallp.tile([1, 1], F32, tag="rinv", name="rinv")
        _pin(nc.vector.reciprocal(rinv, st["tot_ps"]), pin)
        rinv_ps = psmall.tile([NC_P, 1], F32, tag="ps_b", name="rinv_ps")
        nc.tensor.matmul(rinv_ps, lhsT=ones_col, rhs=rinv, start=True, stop=True)
        rinv_b = smallp.tile([NC_P, 1], F32, tag="rinv_b", name="rinv_b")
        nc.scalar.copy(rinv_b, rinv_ps)
        _softmax_finish(st, rinv_b)

    def gp_softmax_stage1(st, pin=None):
        # row-max, then one gpsimd all-reduce across partitions
        st["rmax"] = smallp.tile([NC_P, 1], F32, tag="rmax", name="rmax")
        _pin(
            nc.vector.tensor_reduce(
                out=st["rmax"], in_=st["scores"], axis=mybir.AxisListType.X,
                op=mybir.AluOpType.max,
            ),
            pin,
        )
        gmax_b = smallp.tile([NC_P, 1], F32, tag="gmax_b", name="gmax_b")
        nc.gpsimd.partition_all_reduce(
            gmax_b, st["rmax"], channels=NC_P, reduce_op=bass_isa.ReduceOp.max
        )
        st["negmax"] = smallp.tile([NC_P, 1], F32, tag="negmax2", name="negmax2")
        nc.scalar.mul(st["negmax"], gmax_b, -1.0)

    def gp_softmax_stage2(st, pin=None):
        # exp with accumulate, then partition-sum on gpsimd
        st["probs"] = scoresp.tile([NC_P, ncols], F32, tag="probs", name="probs")
        ssum = smallp.tile([NC_P, 1], F32, tag="ssum", name="ssum")
        nc.scalar.activation(
            out=st["probs"], in_=st["scores"],
            func=mybir.ActivationFunctionType.Exp,
            bias=st["negmax"], scale=1.0, accum_out=ssum,
        )
        tot_b = smallp.tile([NC_P, 1], F32, tag="tot_b", name="tot_b")
        nc.gpsimd.partition_all_reduce(
            tot_b, ssum, channels=NC_P, reduce_op=bass_isa.ReduceOp.add
        )
        st["tot_b"] = tot_b

    def gp_softmax_stage3(st, pin=None):
        # 1/sum per partition (all partitions hold the same total)
        rinv_b = smallp.tile([NC_P, 1], F32, tag="rinv_b2", name="rinv_b2")
        _pin(nc.vector.reciprocal(rinv_b, st["tot_b"]), pin)
        _softmax_finish(st, rinv_b)

    def _softmax_finish(st, rinv_b):
        pT_ps = psmall.tile([ncols, NC_P], F32, tag="ps_d", name="pT_ps")
        nc.tensor.transpose(pT_ps, st["probs"], ident)
        pT = scoresp.tile([ncols, NC_P], F32, tag="pT", name="pT")
        # normalization fused into the transposed copy (per-partition scale)
        nc.scalar.mul(pT, pT_ps, rinv_b[0:ncols, 0:1])
        # output stores ride the sync/HWDGE ring, which is idle once W has
        # loaded; the gpsimd ring stays dedicated to the enc cast-stream
        nc.sync.dma_start(out=out_ap[st["b"]], in_=pT)

    stages = [pe_softmax_stage1, pe_softmax_stage2, pe_softmax_stage3]
    gp_stages = [gp_softmax_stage1, gp_softmax_stage2, gp_softmax_stage3]

    def do_tile(st, e_sb, u, t, f32_path=False):
        c = u * NT + t
        # Hybrid dot product, engine-balanced. Measured per-tile costs:
        #   STT (fused accum, DVE only)        ~1220 ns DVE (accum ops are
        #                                       capped at 1x even in fp16)
        #   TT fp16 (2x mode) + ACT copy-accum  ~690 ns DVE + ~1700 ACT
        # Neither engine alone keeps up with the ~94 us HBM stream, so
        # ~9/16 columns take the TT+ACT path and ~7/16 the STT path,
        # putting both engines at ~80 us. (tensor_tensor_reduce would
        # fuse at 2x in theory but wedges the core on this HW.)
        if not f32_path and t < (3 if u == 0 else 2):
            prod = prodp.tile([NC_P, H], F16, tag="prod", name="prod")
            nc.vector.tensor_tensor(
                out=prod,
                in0=e_sb[:, t, :],
                in1=vb_sb[st["b"]],
                op=mybir.AluOpType.mult,
            )
            nc.scalar.activation(
                out=sink,
                in_=prod,
                func=mybir.ActivationFunctionType.Copy,
                bias=0.0,
                scale=1.0,
                accum_out=st["scores"][:, c : c + 1],
            )
        else:
            scr = scratch32 if f32_path else scratchp.tile(
                [NC_P, H], F16, tag="scratch", name="scratch")
            nc.vector.scalar_tensor_tensor(
                out=scr,
                in0=e_sb[:, t, :],
                scalar=1.0,
                in1=vb0_32 if f32_path else vb_sb[st["b"]],
                op0=mybir.AluOpType.mult,
                op1=mybir.AluOpType.mult,
                accum_out=st["scores"][:, c : c + 1],
            )

    prev_st = None
    for b in range(BPC):
        st = {"b": b, "scores": scoresp.tile([NC_P, ncols], F32, tag="scores",
                                             name="scores")}
        for u in range(NU):
            if b == 0 and u == 0:
                e_sb = e32_sb  # HWDGE f32 prefetch, f32 STT path
            elif b == 0 and u - 1 < len(prefetched):
                e_sb = prefetched[u - 1]
            elif b == BPC - 1 and u == NU - 1:
                # split the very last supertile so its first tiles land
                # ~1.3 us earlier and the post-stream tail shrinks
                e_sb = encp.tile([NC_P, NT, H], F16, tag="enc", name="e_sb")
                nc.gpsimd.dma_start(
                    out=e_sb[:, 0 : NT // 2, :],
                    in_=enc_ap[b, u, :, 0 : NT // 2, :],
                )
                nc.gpsimd.dma_start(
                    out=e_sb[:, NT // 2 : NT, :],
                    in_=enc_ap[b, u, :, NT // 2 : NT, :],
                )
            else:
                e_sb = encp.tile([NC_P, NT, H], F16, tag="enc", name="e_sb")
                # SWDGE cast-DMA: HBM f32 -> SBUF fp16 inline; runs on the
                # gpsimd ring so it never queues behind the W chunk loads
                nc.gpsimd.dma_start(out=e_sb, in_=enc_ap[b, u])
            for t in range(NT):
                do_tile(st, e_sb, u, t, f32_path=(b == 0 and u == 0))
            # interleave the previous batch's softmax, one stage per
            # supertile, so its cross-engine round-trips overlap the STT
            # stream (explicit pinning measured slower — scheduler does fine)
            if prev_st is not None and u < len(stages):
                stages[u](prev_st)
        prev_st = st
    for f in gp_stages:
        f(prev_st)


def _get_nc():
    if "nc" not in _CACHED:
        nc = _build_bass()
        # Bacc defers register allocation etc. to finalize(); the PJRT run
        # path serializes the module as-is, so legalize it here.
        nc.finalize()
        _CACHED["nc"] = nc
    return _CACHED["nc"]


def run(hidden, encoder_outputs, W, trace=False):
    """Shard, run on 8 cores, gather. Returns (out [B,1,S], BassKernelResults)."""
    from concourse.bass_utils import run_bass_kernel_spmd

    hidden = np.ascontiguousarray(np.asarray(hidden, dtype=np.float32))
    enc = np.ascontiguousarray(np.asarray(encoder_outputs, dtype=np.float32))
    W16 = np.ascontiguousarray(np.asarray(W, dtype=np.float32).astype(np.float16))

    nc = _get_nc()
    in_maps = []
    for i in range(NCORES):
        sl = slice(i * BPC, (i + 1) * BPC)
        # hTp[p, k*BPC+b] = hidden_shard[b, k*128+p]
        hTp = np.ascontiguousarray(
            hidden[sl].T.reshape(KCH, NC_P, BPC).transpose(1, 0, 2).reshape(
                NC_P, KCH * BPC
            ).astype(np.float16)
        )
        in_maps.append(
            {
                "enc": np.ascontiguousarray(enc[sl]),
                "hTp": hTp,
                "W": W16,
            }
        )
    res = run_bass_kernel_spmd(nc, in_maps, core_ids=list(range(NCORES)), trace=trace)
    out = np.concatenate([r["out"] for r in res.results], axis=0)  # [B, S]
    return out[:, None, :].astype(np.float32), res


def kernel(hidden, encoder_outputs, W, b=None, **_ignored):
    out, _ = run(hidden, encoder_outputs, W)
    return out


# revision 43
# speedup vs baseline: 1.0049x; 1.0049x over previous
"""Trainium2 Bass kernel for nn_Attn_47768626266275.

Computation (reference):
    energy[b,s,:] = W @ enc[b,s,:] + bias          # nn.Linear
    scores[b,s]   = hidden[b,:] . energy[b,s,:]
    out           = softmax(scores, axis=-1)[:, None, :]

Algebraic rewrite used here:
    scores[b,s] = enc[b,s,:] . v[b,:] + c[b],  v = hidden @ W,  c = hidden . bias
    softmax is shift-invariant along s, so c[b] drops out entirely.

This turns the [B*S,H]x[H,H] projection (137 GFLOP) into a [B,H]x[H,H] matmul
plus a streamed per-row dot product -> the kernel is HBM-bound on reading
encoder_outputs exactly once (33.5 MB/core across 8 cores).

Precision: the dot-product stream runs in fp16 (measured end-to-end max rel
err 2.6e-3 vs the fp32 reference, against a 2e-2 gate). enc is cast
f32->fp16 inline by the SWDGE DMA engines during the HBM->SBUF load, which
halves DVE time (tensor_tensor-class ops get the 2x_1p packed mode only for
16-bit dtypes) and halves SBUF footprint; HBM read traffic is unchanged.
The accumulator (scores) stays fp32.

Sharding: data-parallel over batch. Core i handles batches [4i, 4i+4).
No collectives (a W-shard + ReduceScatter of partial v was evaluated and
rejected: the collective's latency sits on the vb critical path that gates
all compute, costing more than the 1.75 MB of W traffic it saves).

Per-core pipeline:
  - SWDGE (gpsimd ring): W fp16 first (same-queue FIFO lands its 2 MB
    ahead of the enc bytes; on the HWDGE ring W gets starved to ~120-190
    GB/s by SWDGE packet competition and vb, which gates ALL compute,
    arrives ~25 us late), then the enc stream in [128, 4, 1024]
    supertiles with inline f32->fp16 cast. Nothing with a cross-engine
    wait may sit on the Pool queue while the stream is live - it stalls
    descriptor generation and with it the whole HBM stream.
  - HWDGE (sync ring): hiddenT, the first supertile as f32 (an all-STT
    f32 path computes it while the Pool queue spools up); output stores.
  - PE: v = hiddenT.T @ W (fp16 in, f32 PSUM); per-batch broadcast of
    v[b] to all 128 partitions via a one-hot stationary matrix.
  - Dot products, hybrid across engines (measured per-[128,1024]-tile:
    STT w/ fused accum = ~1220 ns DVE regardless of dtype - every
    accum-bearing DVE op is capped at 1x mode; plain TT fp16 hits 2x =
    ~690 ns; ACT Copy-with-accum = ~1150+280 ns). Neither engine alone
    keeps up with the ~94 us stream, so ~9/16 columns per batch run
    TT(DVE) + Copy-accum(ACT) and ~7/16 run STT(DVE), putting both
    engines at ~75 us. tensor_tensor_reduce would fuse at 2x on paper
    but wedges the core on this HW (custom-DVE op, no 2x uop either).
  - softmax over the [128, 16] f32 score tile per batch: while the
    stream is live, partition reductions go through PE transpose/
    ones-matmul round-trips (keeps the Pool queue clean); the final
    batch - whose softmax is the exposed serial tail - uses single
    gpsimd partition_all_reduce ops instead, roughly halving the chain.
    The last supertile's DMA is split in half so its tiles land early.

Single-run HW times vary up to ~25% with neighbor traffic on the shared
device; judge changes by the min over >=3 runs (best seen: ~106.7 us,
from a 131 us starting point; floor is ~100: 35.6 MB at ~375 GB/s + ~6.5
us fixed engine-init preamble + ~4 us tail).
"""

import numpy as np

import concourse.bass as bass
import concourse.bacc as bacc
import concourse.tile as tile
from concourse import mybir
from concourse.masks import make_identity

B = 32          # full batch
S = 2048        # sequence
H = 1024        # hidden
NCORES = 8
BPC = B // NCORES   # batches per core = 4
NU = 4          # supertiles per batch (1 MB fp16 each)
NT = 4          # 128-row subtiles per supertile
NC_P = 128      # partitions
KCH = H // NC_P  # 8 contraction chunks for the v matmul

F32 = mybir.dt.float32
F16 = mybir.dt.float16

_CACHED = {}


def _build_bass():
    from contextlib import ExitStack

    nc = bacc.Bacc()

    enc_h = nc.declare_dram_parameter("enc", [BPC, S, H], F32, isOutput=False)
    # hTp[p, k*BPC + b] = hidden[b, k*128 + p] — one contiguous row per
    # partition so the DMA is 128 fat descriptors instead of 1024 tiny ones
    hT_h = nc.declare_dram_parameter("hTp", [NC_P, KCH * BPC], F16, isOutput=False)
    w_h = nc.declare_dram_parameter("W", [H, H], F16, isOutput=False)
    out_h = nc.declare_dram_parameter("out", [BPC, S], F32, isOutput=True)

    with tile.TileContext(nc) as tc, ExitStack() as ctx:
        _emit(ctx, tc, enc_h, hT_h, w_h, out_h)
    return nc


def _emit(ctx, tc, enc_h, hT_h, w_h, out_h):
    nc = tc.nc

    singles = ctx.enter_context(tc.tile_pool(name="singles", bufs=1))
    wchunks = ctx.enter_context(tc.tile_pool(name="wchunks", bufs=8))
    encp = ctx.enter_context(tc.tile_pool(name="encp", bufs=10))
    scratchp = ctx.enter_context(tc.tile_pool(name="scratchp", bufs=3))
    prodp = ctx.enter_context(tc.tile_pool(name="prodp", bufs=5))
    scoresp = ctx.enter_context(tc.tile_pool(name="scoresp", bufs=3))
    smallp = ctx.enter_context(tc.tile_pool(name="smallp", bufs=4))
    pmm = ctx.enter_context(tc.tile_pool(name="pmm", bufs=2, space="PSUM"))
    psmall = ctx.enter_context(tc.tile_pool(name="psmall", bufs=1, space="PSUM"))

    # ---- prefetch the first two enc supertiles ---------------------------
    # issued before any other Pool-queue work (constants below run on
    # gpsimd) so SWDGE descriptor generation — and with it the HBM
    # stream — starts at t~0 instead of ~8 us in
    enc_ap = enc_h[:].rearrange("b (u t p) h -> b u p t h", u=NU, t=NT, p=NC_P)
    # W rides the SWDGE/Pool queue AHEAD of the enc stream. W gates vb which
    # gates all compute, and on the HWDGE ring it gets starved down to
    # ~120-190 GB/s by SWDGE packet competition (compute then starts ~30 us
    # in). Same-queue FIFO guarantees W's 2 MB lands before any enc byte.
    # These are plain fp16 loads (no cast) with no cross-engine waits, so
    # they cannot stall descriptor generation.
    w_ap = w_h[:].rearrange("(k p) h -> k p h", p=NC_P)
    w_sbs = []
    for k in range(KCH):
        w_sb = wchunks.tile([NC_P, H], F16, tag="w", name="w_sb")
        nc.gpsimd.dma_start(out=w_sb, in_=w_ap[k])
        w_sbs.append(w_sb)
    prefetched = []
    for u in range(1, 3):
        e_sb = encp.tile([NC_P, NT, H], F16, tag="enc", name="e_sb")
        nc.gpsimd.dma_start(out=e_sb, in_=enc_ap[0, u])
        prefetched.append(e_sb)

    # ---- constants -------------------------------------------------------
    ident = singles.tile([NC_P, NC_P], F32, tag="ident")
    make_identity(nc, ident)
    ones_col = singles.tile([1, NC_P], F32, tag="ones_col")   # lhsT for bcast
    nc.vector.memset(ones_col, 1.0)
    ones_sum = singles.tile([NC_P, 1], F32, tag="ones_sum")   # rhs for P-sum
    nc.vector.memset(ones_sum, 1.0)
    # junk tile for PE warmup — DVE memset, ready ~4 us before make_identity
    # (which sits behind the Q7 preamble + prefetch descriptor generation)
    warm_in = singles.tile([NC_P, NC_P], F32, tag="warm_in")
    nc.vector.memset(warm_in, 0.0)
    # sel[:, b, :] is a [BPC, 128] stationary matrix whose row b is all-ones:
    # matmul(lhsT=sel[:,b,:], rhs=v_sb) broadcasts v[b,:] to all partitions.
    sel = singles.tile([BPC, BPC, NC_P], F16, tag="sel")
    nc.gpsimd.memset(sel, 0.0)
    nc.gpsimd.affine_select(
        out=sel,
        in_=sel,
        compare_op=mybir.AluOpType.not_equal,
        fill=1.0,
        base=0,
        # expr = p - b  -> fill 1.0 where p == b
        pattern=[[-1, BPC], [0, NC_P]],
        channel_multiplier=1,
    )

    # ---- PE warmup: ~3.5 us of junk matmuls so the HAM clock-gate opens
    # (1.2 -> 2.4 GHz) before the v-chain matmuls arrive
    warm_ps = pmm.tile([NC_P, NC_P], F32, tag="mm", name="warm_ps")
    for _ in range(8):
        nc.tensor.matmul(warm_ps, lhsT=warm_in, rhs=warm_in, start=True, stop=True)

    # ---- load packed hiddenT (tiny, first on the sync ring) -------------
    hT_sb = singles.tile([NC_P, KCH, BPC], F16, tag="hT_sb")
    nc.sync.dma_start(
        out=hT_sb, in_=hT_h[:].rearrange("p (k b) -> p k b", b=BPC)
    )

    # ---- first supertile (b0,u0) rides the now-idle HWDGE ring as f32 ---
    # with W moved to the Pool queue, HWDGE has nothing but hT + output
    # stores; these 2 MB stream concurrently with W and come off the tail
    # of the SWDGE stream. The 4 tiles use an all-STT f32 path (f32 STT
    # costs the same ~1220 ns as fp16).
    e32_sb = singles.tile([NC_P, NT, H], F32, tag="e32_sb")
    nc.sync.dma_start(out=e32_sb, in_=enc_ap[0, 0])
    scratch32 = singles.tile([NC_P, H], F32, tag="scratch32")

    # ---- v = hiddenT.T @ W, chunk matmuls chase the W chunk arrivals ----
    v_ps = pmm.tile([BPC, H], F32, tag="mm")
    for k in range(KCH):
        for half in range(2):
            cols = slice(half * 512, (half + 1) * 512)
            nc.tensor.matmul(
                v_ps[:, cols],
                lhsT=hT_sb[:, k, :],
                rhs=w_sbs[k][:, cols],
                start=(k == 0),
                stop=(k == KCH - 1),
            )
    v_sb = singles.tile([BPC, H], F16, tag="v_sb")
    nc.scalar.copy(v_sb, v_ps)

    # ---- broadcast v[b] across all 128 partitions -----------------------
    vb_sb = []
    for b in range(BPC):
        vb_ps = pmm.tile([NC_P, H], F32, tag="mm")
        for half in range(2):
            cols = slice(half * 512, (half + 1) * 512)
            nc.tensor.matmul(
                vb_ps[:, cols],
                lhsT=sel[:, b, :],
                rhs=v_sb[:, cols],
                start=True,
                stop=True,
            )
        t = singles.tile([NC_P, H], F16, tag=f"vb{b}")
        nc.scalar.copy(t, vb_ps)
        vb_sb.append(t)
        if b == 0:
            vb0_32 = singles.tile([NC_P, H], F32, tag="vb0_32")
            nc.scalar.copy(vb0_32, vb_ps)

    # ---- main stream: scores + softmax ----------------------------------
    out_ap = out_h[:].rearrange("b (c p) -> b c p", p=NC_P)  # c = u*NT + t
    ncols = NU * NT
    # ACT's main output for the accumulate pass; the data is discarded
    # (only accum_out matters) so one buffer is enough — ACT is in-order
    sink = singles.tile([NC_P, H], F16, tag="sink")

    from concourse.tile import add_dep_helper

    def _pin(op, pin):
        # order a softmax DVE op after the given STT in the DVE stream so the
        # in-order DVE never idles on the op's cross-engine dependencies
        if pin is not None:
            add_dep_helper(op.ins, pin.ins, sync=False,
                           reason="defer softmax DVE op behind STT stream")

    from concourse import bass_isa

    # Two softmax implementations. The PE-based one is used while the enc
    # stream is live: its cross-engine round-trips hide under the stream and
    # it keeps the gpsimd/Pool queue free (any cross-engine wait there stalls
    # SWDGE descriptor generation and with it the whole HBM stream). The
    # gpsimd-based one collapses the partition reductions into single Q7 ops
    # and is used only for the final batch, after the last enc DMA has been
    # issued — it roughly halves the serial tail chain.

    def pe_softmax_stage1(st, pin=None):
        # row-max over the 16 score columns, transpose to one partition
        st["rmax"] = smallp.tile([NC_P, 1], F32, tag="rmax", name="rmax")
        _pin(
            nc.vector.tensor_reduce(
                out=st["rmax"], in_=st["scores"], axis=mybir.AxisListType.X,
                op=mybir.AluOpType.max,
            ),
            pin,
        )
        rmaxT_ps = psmall.tile([1, NC_P], F32, tag="ps_a", name="rmaxT_ps")
        nc.tensor.transpose(rmaxT_ps, st["rmax"], ident)
        st["rmaxT"] = smallp.tile([1, NC_P], F32, tag="rmaxT", name="rmaxT")
        nc.scalar.copy(st["rmaxT"], rmaxT_ps)

    def pe_softmax_stage2(st, pin=None):
        # global max -> -max on all partitions -> exp with accumulate -> sum
        gmax = smallp.tile([1, 1], F32, tag="gmax", name="gmax")
        _pin(
            nc.vector.tensor_reduce(
                out=gmax, in_=st["rmaxT"], axis=mybir.AxisListType.X,
                op=mybir.AluOpType.max,
            ),
            pin,
        )
        gmax_ps = psmall.tile([NC_P, 1], F32, tag="ps_b", name="gmax_ps")
        nc.tensor.matmul(gmax_ps, lhsT=ones_col, rhs=gmax, start=True, stop=True)
        negmax = smallp.tile([NC_P, 1], F32, tag="negmax", name="negmax")
        nc.scalar.mul(negmax, gmax_ps, -1.0)
        st["probs"] = scoresp.tile([NC_P, ncols], F32, tag="probs", name="probs")
        ssum = smallp.tile([NC_P, 1], F32, tag="ssum", name="ssum")
        nc.scalar.activation(
            out=st["probs"], in_=st["scores"],
            func=mybir.ActivationFunctionType.Exp,
            bias=negmax, scale=1.0, accum_out=ssum,
        )
        tot_ps = psmall.tile([1, 1], F32, tag="ps_c", name="tot_ps")
        nc.tensor.matmul(tot_ps, lhsT=ssum, rhs=ones_sum, start=True, stop=True)
        st["tot_ps"] = tot_ps

    def pe_softmax_stage3(st, pin=None):
        # 1/sum, broadcast, transposed normalize, store
        rinv = smallp.tile([1, 1], F32, tag="rinv", name="rinv")
        _pin(nc.vector.reciprocal(rinv, st["tot_ps"]), pin)
        rinv_ps = psmall.tile([NC_P, 1], F32, tag="ps_b", name="rinv_ps")
        nc.tensor.matmul(rinv_ps, lhsT=ones_col, rhs=rinv, start=True, stop=True)
        rinv_b = smallp.tile([NC_P, 1], F32, tag="rinv_b", name="rinv_b")
        nc.scalar.copy(rinv_b, rinv_ps)
        _softmax_finish(st, rinv_b)

    def gp_softmax_stage1(st, pin=None):
        # row-max, then one gpsimd all-reduce across partitions
        st["rmax"] = smallp.tile([NC_P, 1], F32, tag="rmax", name="rmax")
        _pin(
            nc.vector.tensor_reduce(
                out=st["rmax"], in_=st["scores"], axis=mybir.AxisListType.X,
                op=mybir.AluOpType.max,
            ),
            pin,
        )
        gmax_b = smallp.tile([NC_P, 1], F32, tag="gmax_b", name="gmax_b")
        nc.gpsimd.partition_all_reduce(
            gmax_b, st["rmax"], channels=NC_P, reduce_op=bass_isa.ReduceOp.max
        )
        st["negmax"] = smallp.tile([NC_P, 1], F32, tag="negmax2", name="negmax2")
        nc.scalar.mul(st["negmax"], gmax_b, -1.0)

    def gp_softmax_stage2(st, pin=None):
        # exp with accumulate, then partition-sum on gpsimd
        st["probs"] = scoresp.tile([NC_P, ncols], F32, tag="probs", name="probs")
        ssum = smallp.tile([NC_P, 1], F32, tag="ssum", name="ssum")
        nc.scalar.activation(
            out=st["probs"], in_=st["scores"],
            func=mybir.ActivationFunctionType.Exp,
            bias=st["negmax"], scale=1.0, accum_out=ssum,
        )
        tot_b = smallp.tile([NC_P, 1], F32, tag="tot_b", name="tot_b")
        nc.gpsimd.partition_all_reduce(
            tot_b, ssum, channels=NC_P, reduce_op=bass_isa.ReduceOp.add
        )
        st["tot_b"] = tot_b

    def gp_softmax_stage3(st, pin=None):
        # 1/sum per partition (all partitions hold the same total)
        rinv_b = smallp.tile([NC_P, 1], F32, tag="rinv_b2", name="rinv_b2")
        _pin(nc.vector.reciprocal(rinv_b, st["tot_b"]), pin)
        _softmax_finish(st, rinv_b)

    def _softmax_finish(st, rinv_b):
        pT_ps = psmall.tile([ncols, NC_P], F32, tag="ps_d", name="pT_ps")
        nc.tensor.transpose(pT_ps, st["probs"], ident)
        pT = scoresp.tile([ncols, NC_P], F32, tag="pT", name="pT")
        # normalization fused into the transposed copy (per-partition scale)
        nc.scalar.mul(pT, pT_ps, rinv_b[0:ncols, 0:1])
        # output stores ride the sync/HWDGE ring, which is idle once W has
        # loaded; the gpsimd ring stays dedicated to the enc cast-stream
        nc.sync.dma_start(out=out_ap[st["b"]], in_=pT)

    stages = [pe_softmax_stage1, pe_softmax_stage2, pe_softmax_stage3]
    gp_stages = [gp_softmax_stage1, gp_softmax_stage2, gp_softmax_stage3]

    def do_tile(st, e_sb, u, t, f32_path=False):
        c = u * NT + t
        # Hybrid dot product, engine-balanced. Measured per-tile costs:
        #   STT (fused accum, DVE only)        ~1220 ns DVE (accum ops are
        #                                       capped at 1x even in fp16)
        #   TT fp16 (2x mode) + ACT copy-accum  ~690 ns DVE + ~1700 ACT
        # Neither engine alone keeps up with the ~94 us HBM stream, so
        # ~9/16 columns take the TT+ACT path and ~7/16 the STT path,
        # putting both engines at ~80 us. (tensor_tensor_reduce would
        # fuse at 2x in theory but wedges the core on this HW.)
        if not f32_path and t < (3 if u == 0 else 2):
            prod = prodp.tile([NC_P, H], F16, tag="prod", name="prod")
            nc.vector.tensor_tensor(
                out=prod,
                in0=e_sb[:, t, :],
                in1=vb_sb[st["b"]],
                op=mybir.AluOpType.mult,
            )
            nc.scalar.activation(
                out=sink,
                in_=prod,
                func=mybir.ActivationFunctionType.Copy,
                bias=0.0,
                scale=1.0,
                accum_out=st["scores"][:, c : c + 1],
            )
        else:
            scr = scratch32 if f32_path else scratchp.tile(
                [NC_P, H], F16, tag="scratch", name="scratch")
            nc.vector.scalar_tensor_tensor(
                out=scr,
                in0=e_sb[:, t, :],
                scalar=1.0,
                in1=vb0_32 if f32_path else vb_sb[st["b"]],
                op0=mybir.AluOpType.mult,
                op1=mybir.AluOpType.mult,
                accum_out=st["scores"][:, c : c + 1],
            )

    prev_st = None
    for b in range(BPC):
        st = {"b": b, "scores": scoresp.tile([NC_P, ncols], F32, tag="scores",
                                             name="scores")}
        for u in range(NU):
            if b == 0 and u == 0:
                e_sb = e32_sb  # HWDGE f32 prefetch, f32 STT path
            elif b == 0 and u - 1 < len(prefetched):
                e_sb = prefetched[u - 1]
            elif b == BPC - 1 and u == NU - 1:
                # split the very last supertile so its first tiles land
                # ~1.3 us earlier and the post-stream tail shrinks
                e_sb = encp.tile([NC_P, NT, H], F16, tag="enc", name="e_sb")
                nc.gpsimd.dma_start(
                    out=e_sb[:, 0 : NT // 2, :],
                    in_=enc_ap[b, u, :, 0 : NT // 2, :],
                )
                nc.gpsimd.dma_start(
                    out=e_sb[:, NT // 2 : NT, :],
                    in_=enc_ap[b, u, :, NT // 2 : NT, :],
                )
            else:
                e_sb = encp.tile([NC_P, NT, H], F16, tag="enc", name="e_sb")
                # SWDGE cast-DMA: HBM f32 -> SBUF fp16 inline; runs on the
                # gpsimd ring so it never queues behind the W chunk loads
                nc.gpsimd.dma_start(out=e_sb, in_=enc_ap[b, u])
            for t in range(NT):
                do_tile(st, e_sb, u, t, f32_path=(b == 0 and u == 0))
            # interleave the previous batch's softmax, one stage per
            # supertile, so its cross-engine round-trips overlap the STT
            # stream (explicit pinning measured slower — scheduler does fine)
            if prev_st is not None and u < len(stages):
                stages[u](prev_st)
        prev_st = st
    for f in gp_stages:
        f(prev_st)


def _get_nc():
    if "nc" not in _CACHED:
        nc = _build_bass()
        # Bacc defers register allocation etc. to finalize(); the PJRT run
        # path serializes the module as-is, so legalize it here.
        nc.finalize()
        _CACHED["nc"] = nc
    return _CACHED["nc"]


def run(hidden, encoder_outputs, W, trace=False):
    """Shard, run on 8 cores, gather. Returns (out [B,1,S], BassKernelResults)."""
    from concourse.bass_utils import run_bass_kernel_spmd

    hidden = np.ascontiguousarray(np.asarray(hidden, dtype=np.float32))
    enc = np.ascontiguousarray(np.asarray(encoder_outputs, dtype=np.float32))
    W16 = np.ascontiguousarray(np.asarray(W, dtype=np.float32).astype(np.float16))

    nc = _get_nc()
    in_maps = []
    for i in range(NCORES):
        sl = slice(i * BPC, (i + 1) * BPC)
        # hTp[p, k*BPC+b] = hidden_shard[b, k*128+p]
        hTp = np.ascontiguousarray(
            hidden[sl].T.reshape(KCH, NC_P, BPC).transpose(1, 0, 2).reshape(
                NC_P, KCH * BPC
            ).astype(np.float16)
        )
        in_maps.append(
            {
                "enc": np.ascontiguousarray(enc[sl]),
                "hTp": hTp,
                "W": W16,
            }
        )
    res = run_bass_kernel_spmd(nc, in_maps, core_ids=list(range(NCORES)), trace=trace)
    out = np.concatenate([r["out"] for r in res.results], axis=0)  # [B, S]
    return out[:, None, :].astype(np.float32), res


def kernel(hidden, encoder_outputs, W, b=None, **_ignored):
    out, _ = run(hidden, encoder_outputs, W)
    return out
